# revision 15
# baseline (speedup 1.0000x reference)
"""Trainium2 Bass kernel for nn_BasicTransformerBlock_18657337934637.

Sparse-attention transformer block:
  q/k/v = hidden @ W* + b*        (2304 -> 2304, 24 heads x 96)
  RoPE3D on q, k
  sparse-1d grouping (SPARSE_N=4): token t -> group t%4, 1024 tokens/group
  softmax attention within each (group, head)
  out = attn @ wo + bo

Distribution over 8 NeuronCores:
  Launch 1 (head-parallel): core c computes heads 3c..3c+2 end-to-end through
    attention.  Host pre-transposes hidden to hT [2304, 4096] in grouped token
    order and casts everything on the matmul path to bf16 (psum accumulation
    stays fp32; rel-err budget 2e-2 >> bf16's ~5e-3).  Per 128-token sub-tile,
    the 3 projections run as 2 matmuls of 432 columns from a host-packed
    [wq|wv|wk] weight, into one 2-bank psum tile.  Per (group, head): scores
    are computed transposed [k, q]; exp skips the max subtraction (scores are
    O(5)); an all-ones column appended to v yields the softmax denominator in
    the same matmul.  The last group's 6 attention instances drain after the
    final projection with nothing to overlap, so they reuse the then-idle
    2-bank QKV psum slots as 1024-wide score tiles, halving the per-ACTIVATE
    overhead on the scalar engine (the tail is exp-throughput-bound).
    Output: un-normalized attn^T + denominator row per (group, head, q-half),
    [24, 97, 512] bf16 contiguous; the host divides.
  Host: gather heads -> attnT [2304, 4096] bf16, undo token permutation.
  Launch 2 (token x outdim parallel): core (i, j) computes
    out[i*1024:(i+1)*1024, j*1152:(j+1)*1152]^T = wo_j^T @ attnT_i
    in bf16 (weight stationary on the PE), with input DMAs interleaved
    across the sync/scalar queues and bf16 outputs on the gpsimd queue.
"""
import os
import numpy as np

HEADS = 24
HD = 96
SPN = 4
S = 4096
DIM = 2304
KC = DIM // 128            # 18 contraction chunks
HPC = 3                    # heads per core
CW = HPC * HD              # 288 columns per core per projection
PW = 3 * CW                # 864 packed projection columns
G = S // SPN               # 1024 tokens per group
TB = 256                   # hT dma block (tokens)
NB = S // TB               # 16 blocks
WG = 6                     # weight chunk-groups (3 kc each)
SCALE = 1.0 / float(np.sqrt(HD))

_CACHE = {}
LAST_RESULTS = []          # test harness introspection


def _build_launch1():
    import concourse.mybir as mybir
    import concourse.tile as tile
    from concourse import bacc
    from concourse.masks import make_identity

    f32 = mybir.dt.float32
    bf16 = mybir.dt.bfloat16
    Exp = mybir.ActivationFunctionType.Exp
    MUL = mybir.AluOpType.mult
    ADD = mybir.AluOpType.add
    nc = bacc.Bacc("TRN2", target_bir_lowering=False, debug=False)

    # all inputs host-pre-tiled to the exact SBUF layouts -> every DMA is a
    # plain 2D copy with multi-KB contiguous rows (full HBM bandwidth)
    hT_d = nc.dram_tensor("hT", [NB, 128, KC * TB], bf16,
                          kind="ExternalInput").ap()
    w_d = nc.dram_tensor("w", [128, KC * PW], bf16, kind="ExternalInput").ap()
    # merged rope tables [A | B] and biases [bq | bk | bvi] (pre-replicated)
    AB_d = nc.dram_tensor("AB", [NB, 128, 4 * CW], bf16,
                          kind="ExternalInput").ap()
    bias_d = nc.dram_tensor("bias", [128, 2 * CW + HPC * (HD + 1)], bf16,
                            kind="ExternalInput").ap()
    # per-instance contiguous output: slot = g*6 + h*2 + qh
    outN_d = nc.dram_tensor("outN", [SPN * HPC * 2, HD + 1, 512], bf16,
                            kind="ExternalOutput").ap()

    with tile.TileContext(nc) as tc:
        with (
            tc.tile_pool(name="singles", bufs=1) as singles,
            tc.tile_pool(name="hp", bufs=2) as hp,
            tc.tile_pool(name="rp", bufs=3) as rp,
            tc.tile_pool(name="qkp", bufs=3) as qkp,
            tc.tile_pool(name="qrp", bufs=3) as qrp,
            tc.tile_pool(name="vp", bufs=16) as vp,
            tc.tile_pool(name="qtp", bufs=2) as qtp,
            tc.tile_pool(name="ktp", bufs=2) as ktp,
            tc.tile_pool(name="ep", bufs=3) as ep,
            tc.tile_pool(name="op", bufs=3) as op,
            # PSUM budget (8 banks): ps 2x2 + stpt 2x1 + pv 2x1
            tc.tile_pool(name="ppq", bufs=2, space="PSUM") as ppq,
            tc.tile_pool(name="pps", bufs=2, space="PSUM") as pps,
            tc.tile_pool(name="ppv", bufs=2, space="PSUM") as ppv,
        ):
            ident = singles.tile([128, 128], bf16, tag="ident", name="ident")
            make_identity(nc, ident)

            # DMA ring plan (3 independent rings, each ~1/3 of HBM bw):
            #   sync:   wg0, wg2, wg4, then hT blocks 1,3,5,... + outN out
            #   scalar: hT block 0 (thirds), wg1, wg3, wg5, biases,
            #           then hT blocks 2,4,6,...
            #   gpsimd: per-block AB rope tables
            def fetch_blk(blk):
                ht = hp.tile([128, KC * TB], bf16, tag="ht", name=f"ht{blk}")
                if blk == 0:
                    for p in range(3):
                        nc.scalar.dma_start(
                            ht[:, p * 6 * TB:(p + 1) * 6 * TB],
                            hT_d[0][:, p * 6 * TB:(p + 1) * 6 * TB])
                else:
                    eng = nc.sync if blk % 2 == 1 else nc.scalar
                    eng.dma_start(ht, hT_d[blk])
                ab_t = rp.tile([128, 4 * CW], bf16, tag="ab", name=f"ab{blk}")
                nc.gpsimd.dma_start(ab_t, AB_d[blk])
                return ht, ab_t

            _pref = {0: fetch_blk(0)}
            # packed [wq|wv|wk] weights in 6 chunk-groups across all 3 rings
            w_grp = []
            for gi, eng in zip(range(WG), (nc.sync, nc.scalar, nc.gpsimd,
                                           nc.sync, nc.scalar, nc.gpsimd)):
                t = singles.tile([128, (KC // WG) * PW], bf16,
                                 tag=f"w_sb{gi}", name=f"w_sb{gi}")
                eng.dma_start(t, w_d[:, gi * (KC // WG) * PW:
                                     (gi + 1) * (KC // WG) * PW])
                w_grp.append(t.rearrange("p (k c) -> p k c", k=KC // WG))

            def w_kc(kc):
                return w_grp[kc // (KC // WG)][:, kc % (KC // WG), :]

            bias_sb = singles.tile([128, 2 * CW + HPC * (HD + 1)], bf16,
                                   tag="bias_sb", name="bias_sb")
            nc.scalar.dma_start(bias_sb, bias_d)
            bq_sb = bias_sb[:, 0:CW]
            bk_sb = bias_sb[:, CW:2 * CW]
            bvi_sb = bias_sb[:, 2 * CW:]
            ones3 = singles.tile([128, HPC], bf16, tag="ones3", name="ones3")
            nc.vector.memset(ones3, 1.0)

            qT, kT, vt = {}, {}, {}
            pending = []   # attention instances awaiting emission

            def emit_ot(g, h, qh, pv):
                ot = op.tile([HD + 1, 512], bf16, tag="ot",
                             name=f"ot{g}_{h}_{qh}")
                nc.vector.tensor_copy(ot, pv)
                nc.sync.dma_start(outN_d[g * 6 + h * 2 + qh], ot)

            def attn_instance(g, h, qh):
                """scoresT -> exp -> PV for one (group, head, query-half),
                software-pipelined over the 8 key chunks."""
                pv = ppv.tile([HD + 1, 512], f32, tag="pv",
                              name=f"pv{g}_{h}_{qh}")
                qs = qT[g][:, h * G + qh * 512:h * G + (qh + 1) * 512]

                def exp_pv(kc, st):
                    ex = ep.tile([128, 512], bf16, tag="ex",
                                 name=f"ex{g}_{h}_{qh}_{kc}")
                    nc.scalar.activation(ex, st, Exp, scale=SCALE)
                    nc.tensor.matmul(
                        pv, vt[(g, kc)][:, h * 97:(h + 1) * 97], ex,
                        start=(kc == 0), stop=(kc == 7))

                sts = []
                for kc in range(8):
                    st = pps.tile([128, 512], f32, tag="stpt",
                                  name=f"st{g}_{h}_{qh}_{kc}")
                    nc.tensor.matmul(
                        st, kT[g][:, h * G + kc * 128:h * G + (kc + 1) * 128],
                        qs, start=True, stop=True)
                    sts.append(st)
                    if kc >= 1:
                        exp_pv(kc - 1, sts[kc - 1])
                exp_pv(7, sts[7])
                emit_ot(g, h, qh, pv)

            def wide_pair(g, h, qh, pv, kcp):
                """One 1024-wide score pair + exp + PV accumulate, using the
                idle QKV psum slots -> half the ACTIVATE count (the attention
                drain is ACT-bound)."""
                qs = qT[g][:, h * G + qh * 512:h * G + (qh + 1) * 512]
                stw = ppq.tile([128, 1024], f32, tag="ps",
                               name=f"stw{g}_{h}_{qh}_{kcp}")
                for j in (0, 1):
                    kc = kcp * 2 + j
                    nc.tensor.matmul(
                        stw[:, j * 512:(j + 1) * 512],
                        kT[g][:, h * G + kc * 128:h * G + (kc + 1) * 128],
                        qs, start=True, stop=True)
                ex = ep.tile([128, 1024], bf16, tag="ex",
                             name=f"exw{g}_{h}_{qh}_{kcp}")
                nc.scalar.activation(ex, stw, Exp, scale=SCALE)
                for j in (0, 1):
                    kc = kcp * 2 + j
                    nc.tensor.matmul(
                        pv, vt[(g, kc)][:, h * 97:(h + 1) * 97],
                        ex[:, j * 512:(j + 1) * 512],
                        start=(kc == 0), stop=(kc == 7))

            wide_started = {}

            def attn_wide_begin(g, h, qh):
                """First 3 score pairs (key chunks 0-5) -- emitted during the
                final projection block, where ACT is otherwise idle."""
                pv = ppv.tile([HD + 1, 512], f32, tag="pv",
                              name=f"pvw{g}_{h}_{qh}")
                for kcp in range(3):
                    wide_pair(g, h, qh, pv, kcp)
                wide_started[(g, h, qh)] = pv

            def attn_instance_wide(g, h, qh):
                pv = wide_started.pop((g, h, qh), None)
                start_kcp = 3 if pv is not None else 0
                if pv is None:
                    pv = ppv.tile([HD + 1, 512], f32, tag="pv",
                                  name=f"pvw{g}_{h}_{qh}")
                for kcp in range(start_kcp, 4):
                    wide_pair(g, h, qh, pv, kcp)
                emit_ot(g, h, qh, pv)

            def postprocess(tb, g, ps, ab_t):
                """v assembly + rope + per-head transposes for one sub-tile.
                All psum-reading ops come first so the ps slot frees early.
                rope shuffle/mul run on gpsimd, psum evacuations on scalar."""
                sub = tb % 2
                col = (tb % 8) * 128
                a_s = ab_t[:, sub * CW:(sub + 1) * CW]
                b_s = ab_t[:, (2 + sub) * CW:(3 + sub) * CW]
                # V: bias add + interleaved ones column
                # (q = ps[0:288], v = ps[288:432]+ps[512:656], k = ps[656:944])
                v_t = vp.tile([128, HPC * (HD + 1)], bf16, tag="v",
                              name=f"v{tb}")
                vv = v_t.rearrange("p (h c) -> p h c", h=HPC)
                bvv = bvi_sb.rearrange("p (h c) -> p h c", h=HPC)
                nc.vector.tensor_tensor(vv[:, 0, 0:96], ps[:, 288:384],
                                        bvv[:, 0, 0:96], ADD)
                nc.vector.tensor_tensor(vv[:, 1, 0:48], ps[:, 384:432],
                                        bvv[:, 1, 0:48], ADD)
                nc.vector.tensor_tensor(vv[:, 1, 48:96], ps[:, 512:560],
                                        bvv[:, 1, 48:96], ADD)
                nc.vector.tensor_tensor(vv[:, 2, 0:96], ps[:, 560:656],
                                        bvv[:, 2, 0:96], ADD)
                nc.vector.tensor_copy(
                    vv[:, :, 96:97],
                    ones3.rearrange("p (h c) -> p h c", h=HPC))
                vt[(g, tb % 8)] = v_t
                # Q, K bias adds (the remaining psum readers)
                sbs = {}
                for d, src0, bias in (("q", 0, bq_sb), ("k", 656, bk_sb)):
                    q_sb = qkp.tile([128, CW], bf16, tag=f"{d}sb",
                                    name=f"{d}sb{tb}")
                    nc.vector.tensor_tensor(q_sb, ps[:, src0:src0 + CW],
                                            bias, ADD)
                    sbs[d] = q_sb
                # rope + transpose per projection
                for d in ("q", "k"):
                    q_sb = sbs[d]
                    shf = qkp.tile([128, CW], bf16, tag="shf",
                                   name=f"shf_{d}{tb}")
                    qv = q_sb.rearrange("p (c u f) -> p c u f", c=9, u=2)
                    sv = shf.rearrange("p (c u f) -> p c u f", c=9, u=2)
                    nc.vector.tensor_copy(sv[:, :, 0:1, :], qv[:, :, 1:2, :])
                    nc.vector.tensor_copy(sv[:, :, 1:2, :], qv[:, :, 0:1, :])
                    qr = qrp.tile([128, CW], bf16, tag="qr",
                                  name=f"qr_{d}{tb}")
                    nc.vector.tensor_tensor(shf, shf, b_s, MUL)
                    nc.vector.tensor_tensor(q_sb, q_sb, a_s, MUL)
                    nc.vector.tensor_tensor(qr, q_sb, shf, ADD)
                    dst = qT if d == "q" else kT
                    pt3 = pps.tile([HD, HPC * 128], bf16, tag="stpt",
                                   name=f"pt_{d}{tb}")
                    for h in range(HPC):
                        nc.tensor.transpose(
                            pt3[:, h * 128:(h + 1) * 128],
                            qr[:, h * 96:(h + 1) * 96], ident)
                    nc.scalar.copy(
                        dst[g].rearrange("d (h t) -> d h t", h=HPC)
                        [:, :, col:col + 128],
                        pt3.rearrange("d (h t) -> d h t", h=HPC))

            for blk in range(NB):
                g = blk // 4
                if blk % 4 == 0:
                    qT[g] = qtp.tile([HD, HPC * G], bf16, tag="qT",
                                     name=f"qT{g}")
                    kT[g] = ktp.tile([HD, HPC * G], bf16, tag="kT",
                                     name=f"kT{g}")
                ht, ab_t = _pref.pop(blk)
                if blk + 1 < NB:
                    _pref[blk + 1] = fetch_blk(blk + 1)
                htv = ht.rearrange("p (k t) -> p k t", k=KC)

                if blk == 0:
                    # kc-outer over both sub-tiles so weight-chunk consumption
                    # tracks the DMA arrival rate instead of outrunning it
                    pair = [ppq.tile([128, 1024], f32, tag="ps",
                                     name=f"ps{sub}") for sub in range(2)]
                    for kc in range(KC):
                        for sub in range(2):
                            lhs = htv[:, kc, sub * 128:(sub + 1) * 128]
                            nc.tensor.matmul(
                                pair[sub][:, 0:432], lhs, w_kc(kc)[:, 0:432],
                                start=(kc == 0), stop=(kc == KC - 1))
                            nc.tensor.matmul(
                                pair[sub][:, 512:944], lhs,
                                w_kc(kc)[:, 432:864],
                                start=(kc == 0), stop=(kc == KC - 1))
                    for sub in range(2):
                        postprocess(sub, g, pair[sub], ab_t)
                    continue

                for sub in range(2):
                    tb = blk * 2 + sub
                    ps = ppq.tile([128, 1024], f32, tag="ps", name=f"ps{tb}")
                    for kc in range(KC):
                        lhs = htv[:, kc, sub * 128:(sub + 1) * 128]
                        nc.tensor.matmul(ps[:, 0:432], lhs, w_kc(kc)[:, 0:432],
                                         start=(kc == 0), stop=(kc == KC - 1))
                        nc.tensor.matmul(ps[:, 512:944], lhs,
                                         w_kc(kc)[:, 432:864],
                                         start=(kc == 0), stop=(kc == KC - 1))
                    postprocess(tb, g, ps, ab_t)
                    # drain one pending attention instance per sub-tile
                    if pending:
                        attn_instance(*pending.pop(0))
                    elif blk == NB - 1 and sub == 0:
                        # last block: pre-start the final group's qh0
                        # attention for key chunks 0-5 (ACT is idle here;
                        # those kT/vt chunks landed during blocks 12-14)
                        attn_wide_begin(g, 0, 0)
                        attn_wide_begin(g, 1, 0)
                if blk % 4 == 3 and blk != NB - 1:
                    pending.extend((g, h, qh)
                                   for h in range(HPC) for qh in range(2))
            gl = SPN - 1
            for h in range(2):
                attn_instance_wide(gl, h, 0)
            attn_instance_wide(gl, 2, 0)
            for h in range(HPC):
                attn_instance_wide(gl, h, 1)
    nc.compile()
    return nc


def _build_launch2():
    import concourse.mybir as mybir
    import concourse.tile as tile
    from concourse import bacc

    f32 = mybir.dt.float32
    bf16 = mybir.dt.bfloat16
    TOK = 1024           # tokens per core
    NW = 1152            # outdims per core
    MB = NW // 128       # 9 outdim blocks
    nc = bacc.Bacc("TRN2", target_bir_lowering=False, debug=False)

    at_d = nc.dram_tensor("attnT", [DIM, TOK], bf16, kind="ExternalInput").ap()
    wo_d = nc.dram_tensor("woj", [DIM, NW], bf16, kind="ExternalInput").ap()
    bo_d = nc.dram_tensor("boj", [1, NW], f32, kind="ExternalInput").ap()
    # transposed output [outdim, tok]; host transposes back
    out_d = nc.dram_tensor("out", [NW, TOK], bf16, kind="ExternalOutput").ap()

    with tile.TileContext(nc) as tc:
        ats, wos = [], []
        with (
            tc.tile_pool(name="singles2", bufs=1) as singles,
            tc.tile_pool(name="atp", bufs=KC) as atp,
            tc.tile_pool(name="wop", bufs=KC) as wop,
            tc.tile_pool(name="outp", bufs=4) as outp,
            tc.tile_pool(name="psp", bufs=8, space="PSUM") as psp,
        ):
            bo_sb = singles.tile([128, MB], f32, tag="bo_sb", name="bo_sb")
            nc.gpsimd.dma_start(bo_sb,
                                bo_d.rearrange("a (m p) -> p (a m)", p=128))
            # input chunks interleaved across two queues in kc order; chunk 0
            # split in halves so the first matmul group starts sooner
            for kc in range(KC):
                a = atp.tile([128, TOK], bf16, tag="at", name=f"at{kc}")
                if kc == 0:
                    nc.sync.dma_start(a[:, 0:512], at_d[0:128, 0:512])
                    nc.sync.dma_start(a[:, 512:TOK], at_d[0:128, 512:TOK])
                else:
                    nc.sync.dma_start(a, at_d[kc * 128:(kc + 1) * 128, :])
                ats.append(a)
                w = wop.tile([128, NW], bf16, tag="wo", name=f"wo{kc}")
                if kc == 0:
                    nc.scalar.dma_start(w[:, 0:576], wo_d[0:128, 0:576])
                    nc.scalar.dma_start(w[:, 576:NW], wo_d[0:128, 576:NW])
                else:
                    nc.scalar.dma_start(w, wo_d[kc * 128:(kc + 1) * 128, :])
                wos.append(w)
            # chunk-outer accumulation over groups of 4 outdim blocks
            # (8 psum banks per group) so the PE tracks the DMA feed instead
            # of serializing behind it.
            units = [(mb, th) for mb in range(MB) for th in range(2)]
            ots = {}
            for base in range(0, len(units), 8):
                grp = units[base:base + 8]
                pss = {}
                for mb, th in grp:
                    pss[(mb, th)] = psp.tile([128, 512], f32, tag="ps",
                                             name=f"ps{mb}_{th}")
                for kc in range(KC):
                    for mb, th in grp:
                        nc.tensor.matmul(
                            pss[(mb, th)], wos[kc][:, mb * 128:(mb + 1) * 128],
                            ats[kc][:, th * 512:(th + 1) * 512],
                            start=(kc == 0), stop=(kc == KC - 1))
                for mb, th in grp:
                    if mb not in ots:
                        ots[mb] = outp.tile([128, TOK], bf16, tag="ot",
                                            name=f"ot{mb}")
                    nc.vector.tensor_scalar_add(
                        ots[mb][:, th * 512:(th + 1) * 512], pss[(mb, th)],
                        bo_sb[:, mb:mb + 1])
                    if th == 1:
                        nc.gpsimd.dma_start(out_d[mb * 128:(mb + 1) * 128, :],
                                            ots[mb])
    nc.compile()
    return nc


def _get(name, builder):
    if name not in _CACHE:
        _CACHE[name] = builder()
    return _CACHE[name]


def _rope_tables(frame, height, width):
    t = np.repeat(np.arange(frame), height * width)
    y = np.tile(np.repeat(np.arange(height), width), frame)
    x = np.tile(np.arange(width), frame * height)
    D = HD // 3
    A = np.empty((S, HD), np.float32)
    B = np.empty((S, HD), np.float32)
    for i, pos in enumerate((t, y, x)):
        inv = 1.0 / (10000.0 ** (np.arange(0, D, 2, dtype=np.float32) / D))
        f = pos[:, None].astype(np.float32) * inv[None, :]
        A[:, i * D:i * D + 16] = np.cos(f)
        A[:, i * D + 16:(i + 1) * D] = np.cos(f)
        B[:, i * D:i * D + 16] = -np.sin(f)
        B[:, i * D + 16:(i + 1) * D] = np.sin(f)
    return A, B


def _tile_hT(hT):
    # [2304, 4096] -> [NB, 128, KC*TB]: blk-major, partition-major, then
    # (chunk, token) contiguous per partition
    return np.ascontiguousarray(
        hT.reshape(KC, 128, NB, TB).transpose(2, 1, 0, 3).reshape(
            NB, 128, KC * TB))


def _tile_w(w):
    # [2304, PW] -> [128, KC*PW]
    return np.ascontiguousarray(
        w.reshape(KC, 128, PW).transpose(1, 0, 2).reshape(128, KC * PW))


def _tile_rope(a):
    # [4096, 288] (pre-tripled) -> [NB, 128, 2*288]
    return np.ascontiguousarray(
        a.reshape(NB, 2, 128, CW).transpose(0, 2, 1, 3).reshape(
            NB, 128, 2 * CW))


def _bias_tensor(bq, bk, bv, sl, bf):
    # [128, 288+288+291] bf16: [bq | bk | bvi] pre-replicated across partitions
    bvi = np.concatenate(
        [np.concatenate([bv[sl][h * HD:(h + 1) * HD], [0.0]])
         for h in range(HPC)]).astype(np.float32)
    row = np.concatenate([bq[sl], bk[sl], bvi]).astype(bf)
    return np.ascontiguousarray(np.broadcast_to(row, (128, row.shape[0])))


def kernel(hidden_states, wq, bq, wk, bk, wv, bv, wo, bo, frame, height, width):
    import ml_dtypes
    from concourse import bass_utils

    bf = ml_dtypes.bfloat16
    f, hh, ww = int(frame), int(height), int(width)
    hs = np.asarray(hidden_states, dtype=np.float32)
    assert hs.shape == (1, S, DIM) and f * hh * ww == S
    wq, wk, wv, wo = (np.asarray(a, np.float32) for a in (wq, wk, wv, wo))
    bq, bk, bv, bo = (np.asarray(a, np.float32) for a in (bq, bk, bv, bo))

    perm = np.concatenate([np.arange(k, S, SPN) for k in range(SPN)])
    A, B = _rope_tables(f, hh, ww)
    A = _tile_rope(np.tile(A[perm], (1, HPC)))
    B = _tile_rope(np.tile(B[perm], (1, HPC)))
    AB = np.concatenate([A, B], axis=2).astype(bf)      # [NB, 128, 4*288]
    hT = _tile_hT(hs[0].T[:, perm]).astype(bf)

    nc1 = _get("l1", _build_launch1)
    in1 = []
    for c in range(8):
        sl = slice(c * CW, (c + 1) * CW)
        wqvk = np.concatenate([wq[:, sl], wv[:, sl], wk[:, sl]], axis=1)
        in1.append({
            "hT": hT,
            "w": _tile_w(wqvk).astype(bf),
            "bias": _bias_tensor(bq, bk, bv, sl, bf),
            "AB": AB,
        })
    td = os.environ.get("KERNEL_TRACE_DIR")
    if td:
        os.makedirs(td + "/l1", exist_ok=True)
        for fn in os.listdir(td + "/l1"):
            os.unlink(td + "/l1/" + fn)
    res1 = bass_utils.run_bass_kernel_spmd(
        nc1, in1, core_ids=list(range(8)),
        tmpdir=(td + "/l1") if td else None)
    LAST_RESULTS.append(res1)

    # outN [24, 97, 512] per core; slot = g*6 + h*2 + qh
    attnT_g = np.empty((DIM, S), np.float32)
    for c in range(8):
        o = np.asarray(res1.results[c]["outN"], dtype=np.float32)
        o = o.reshape(SPN, HPC, 2, HD + 1, 512)
        num = o[:, :, :, :HD, :]                  # [g, h, qh, 96, 512]
        den = o[:, :, :, HD:HD + 1, :]
        a = num / den                             # [g, h, qh, 96, 512]
        # -> rows (3c+h)*96 + d, cols g*1024 + qh*512 + t
        a = a.transpose(1, 3, 0, 2, 4).reshape(HPC * HD, S)
        attnT_g[c * CW:(c + 1) * CW, :] = a
    attnT = np.empty_like(attnT_g)
    attnT[:, perm] = attnT_g

    nc2 = _get("l2", _build_launch2)
    in2 = []
    attnT_bf = attnT.astype(bf)
    for c in range(8):
        i, j = divmod(c, 2)
        in2.append({
            "attnT": np.ascontiguousarray(attnT_bf[:, i * 1024:(i + 1) * 1024]),
            "woj": np.ascontiguousarray(
                wo[:, j * 1152:(j + 1) * 1152].astype(bf)),
            "boj": np.ascontiguousarray(
                bo[j * 1152:(j + 1) * 1152]).reshape(1, 1152),
        })
    if td:
        os.makedirs(td + "/l2", exist_ok=True)
        for fn in os.listdir(td + "/l2"):
            os.unlink(td + "/l2/" + fn)
    res2 = bass_utils.run_bass_kernel_spmd(
        nc2, in2, core_ids=list(range(8)),
        tmpdir=(td + "/l2") if td else None)
    LAST_RESULTS.append(res2)

    out = np.empty((S, DIM), np.float32)
    for c in range(8):
        i, j = divmod(c, 2)
        out[i * 1024:(i + 1) * 1024, j * 1152:(j + 1) * 1152] = \
            np.asarray(res2.results[c]["out"], dtype=np.float32).T
    return out[None]


# revision 16
# speedup vs baseline: 1.0134x; 1.0134x over previous
"""Trainium2 Bass kernel for nn_BasicTransformerBlock_18657337934637.

Sparse-attention transformer block:
  q/k/v = hidden @ W* + b*        (2304 -> 2304, 24 heads x 96)
  RoPE3D on q, k
  sparse-1d grouping (SPARSE_N=4): token t -> group t%4, 1024 tokens/group
  softmax attention within each (group, head)
  out = attn @ wo + bo

Distribution over 8 NeuronCores:
  Launch 1 (head-parallel): core c computes heads 3c..3c+2 end-to-end through
    attention.  Host pre-transposes hidden to hT [2304, 4096] in grouped token
    order and casts everything on the matmul path to bf16 (psum accumulation
    stays fp32; rel-err budget 2e-2 >> bf16's ~7e-3).  Per 128-token sub-tile,
    the 3 projections run as 2 matmuls of 432 columns from a host-packed
    [wq|wv|wk] weight, into one 2-bank psum tile.  Per (group, head): scores
    are computed transposed [k, q]; exp skips the max subtraction (scores are
    O(5)); an all-ones column appended to v yields the softmax denominator in
    the same matmul.  The last group's 6 attention instances drain after the
    final projection with nothing to overlap, so they reuse the then-idle
    2-bank QKV psum slots as 1024-wide score tiles, halving the per-ACTIVATE
    overhead on the scalar engine (the drain is exp-throughput-bound).
    Output: un-normalized attn^T + denominator row per (group, head, q-half),
    [24, 97, 512] bf16 contiguous; the host divides.
  Host: gather heads -> attnT [2304, 4096] bf16, undo token permutation.
  Launch 2 (token x outdim parallel): core (i, j) computes
    out[i*1024:(i+1)*1024, j*1152:(j+1)*1152]^T = wo_j^T @ attnT_i
    in bf16 (weight stationary on the PE), with input DMAs interleaved
    across the sync/scalar queues and bf16 outputs on the gpsimd queue.
"""
import os
import numpy as np

HEADS = 24
HD = 96
SPN = 4
S = 4096
DIM = 2304
KC = DIM // 128            # 18 contraction chunks
HPC = 3                    # heads per core
CW = HPC * HD              # 288 columns per core per projection
PW = 3 * CW                # 864 packed projection columns
G = S // SPN               # 1024 tokens per group
TB = 256                   # hT dma block (tokens)
NB = S // TB               # 16 blocks
WG = 3                     # weight chunk-groups (6 kc each)
SCALE = 1.0 / float(np.sqrt(HD))

_CACHE = {}
LAST_RESULTS = []          # test harness introspection


def _build_launch1():
    import concourse.mybir as mybir
    import concourse.tile as tile
    from concourse import bacc
    from concourse.masks import make_identity

    f32 = mybir.dt.float32
    bf16 = mybir.dt.bfloat16
    Exp = mybir.ActivationFunctionType.Exp
    MUL = mybir.AluOpType.mult
    ADD = mybir.AluOpType.add
    nc = bacc.Bacc("TRN2", target_bir_lowering=False, debug=False)

    # all inputs host-pre-tiled to the exact SBUF layouts -> every DMA is a
    # plain 2D copy with multi-KB contiguous rows (full HBM bandwidth)
    hT_d = nc.dram_tensor("hT", [NB, 128, KC * TB], bf16,
                          kind="ExternalInput").ap()
    w_d = nc.dram_tensor("w", [128, KC * PW], bf16, kind="ExternalInput").ap()
    bq_d = nc.dram_tensor("bq", [1, CW], f32, kind="ExternalInput").ap()
    bk_d = nc.dram_tensor("bk", [1, CW], f32, kind="ExternalInput").ap()
    A_d = nc.dram_tensor("A", [NB, 128, 2 * CW], bf16, kind="ExternalInput").ap()
    B_d = nc.dram_tensor("B", [NB, 128, 2 * CW], bf16, kind="ExternalInput").ap()
    bvi_d = nc.dram_tensor("bvi", [1, HPC * (HD + 1)], f32,
                           kind="ExternalInput").ap()
    # per-instance contiguous output: slot = g*6 + h*2 + qh
    outN_d = nc.dram_tensor("outN", [SPN * HPC * 2, HD + 1, 512], bf16,
                            kind="ExternalOutput").ap()

    with tile.TileContext(nc) as tc:
        with (
            tc.tile_pool(name="singles", bufs=1) as singles,
            tc.tile_pool(name="hp", bufs=2) as hp,
            tc.tile_pool(name="rp", bufs=3) as rp,
            tc.tile_pool(name="qkp", bufs=3) as qkp,
            tc.tile_pool(name="qrp", bufs=3) as qrp,
            tc.tile_pool(name="vp", bufs=16) as vp,
            tc.tile_pool(name="qtp", bufs=2) as qtp,
            tc.tile_pool(name="ktp", bufs=2) as ktp,
            tc.tile_pool(name="ep", bufs=3) as ep,
            tc.tile_pool(name="op", bufs=3) as op,
            # PSUM budget (8 banks): ps 2x2 + stpt 2x1 + pv 2x1
            tc.tile_pool(name="ppq", bufs=2, space="PSUM") as ppq,
            tc.tile_pool(name="pps", bufs=2, space="PSUM") as pps,
            tc.tile_pool(name="ppv", bufs=2, space="PSUM") as ppv,
        ):
            ident = singles.tile([128, 128], bf16, tag="ident", name="ident")
            make_identity(nc, ident)

            # block-0 activations first so the PE can start as soon as the
            # first weight chunk-group lands; block 0 is split into thirds so
            # the first kc chunks gate only on the first third
            def fetch_blk(blk):
                ht = hp.tile([128, KC * TB], bf16, tag="ht", name=f"ht{blk}")
                if blk == 0:
                    for p in range(3):
                        nc.sync.dma_start(
                            ht[:, p * 6 * TB:(p + 1) * 6 * TB],
                            hT_d[0][:, p * 6 * TB:(p + 1) * 6 * TB])
                else:
                    nc.sync.dma_start(ht, hT_d[blk])
                a_t = rp.tile([128, 2 * CW], bf16, tag="a", name=f"a{blk}")
                nc.gpsimd.dma_start(a_t, A_d[blk])
                b_t = rp.tile([128, 2 * CW], bf16, tag="b", name=f"b{blk}")
                nc.gpsimd.dma_start(b_t, B_d[blk])
                return ht, a_t, b_t

            _pref = {0: fetch_blk(0)}
            # packed [wq|wv|wk] weights in 3 chunk-groups across both queues
            w_grp = []
            for gi in range(WG):
                t = singles.tile([128, (KC // WG) * PW], bf16,
                                 tag=f"w_sb{gi}", name=f"w_sb{gi}")
                eng = nc.sync if gi % 2 == 0 else nc.scalar
                eng.dma_start(t, w_d[:, gi * (KC // WG) * PW:
                                     (gi + 1) * (KC // WG) * PW])
                w_grp.append(t.rearrange("p (k c) -> p k c", k=KC // WG))

            def w_kc(kc):
                return w_grp[kc // (KC // WG)][:, kc % (KC // WG), :]

            bq_sb = singles.tile([128, CW], f32, tag="bq_sb", name="bq_sb")
            nc.gpsimd.dma_start(out=bq_sb, in_=bq_d.to_broadcast([128, CW]))
            bk_sb = singles.tile([128, CW], f32, tag="bk_sb", name="bk_sb")
            nc.gpsimd.dma_start(out=bk_sb, in_=bk_d.to_broadcast([128, CW]))
            ones3 = singles.tile([128, HPC], bf16, tag="ones3", name="ones3")
            nc.vector.memset(ones3, 1.0)
            bvi_sb = singles.tile([128, HPC * (HD + 1)], f32, tag="bvi",
                                  name="bvi_sb")
            nc.gpsimd.dma_start(out=bvi_sb,
                                in_=bvi_d.to_broadcast([128, HPC * (HD + 1)]))

            qT, kT, vt = {}, {}, {}
            pending = []   # attention instances awaiting emission

            def emit_ot(g, h, qh, pv):
                ot = op.tile([HD + 1, 512], bf16, tag="ot",
                             name=f"ot{g}_{h}_{qh}")
                nc.vector.tensor_copy(ot, pv)
                nc.sync.dma_start(outN_d[g * 6 + h * 2 + qh], ot)

            def attn_instance(g, h, qh):
                """scoresT -> exp -> PV for one (group, head, query-half),
                software-pipelined over the 8 key chunks."""
                pv = ppv.tile([HD + 1, 512], f32, tag="pv",
                              name=f"pv{g}_{h}_{qh}")
                qs = qT[g][:, h * G + qh * 512:h * G + (qh + 1) * 512]

                def exp_pv(kc, st):
                    ex = ep.tile([128, 512], bf16, tag="ex",
                                 name=f"ex{g}_{h}_{qh}_{kc}")
                    nc.scalar.activation(ex, st, Exp, scale=SCALE)
                    nc.tensor.matmul(
                        pv, vt[(g, kc)][:, h * 97:(h + 1) * 97], ex,
                        start=(kc == 0), stop=(kc == 7))

                sts = []
                for kc in range(8):
                    st = pps.tile([128, 512], f32, tag="stpt",
                                  name=f"st{g}_{h}_{qh}_{kc}")
                    nc.tensor.matmul(
                        st, kT[g][:, h * G + kc * 128:h * G + (kc + 1) * 128],
                        qs, start=True, stop=True)
                    sts.append(st)
                    if kc >= 1:
                        exp_pv(kc - 1, sts[kc - 1])
                exp_pv(7, sts[7])
                emit_ot(g, h, qh, pv)

            def attn_instance_wide(g, h, qh):
                """Tail variant: 1024-wide score tiles in the idle QKV psum
                slots -> half the ACTIVATE count (the tail is ACT-bound)."""
                pv = ppv.tile([HD + 1, 512], f32, tag="pv",
                              name=f"pvw{g}_{h}_{qh}")
                qs = qT[g][:, h * G + qh * 512:h * G + (qh + 1) * 512]

                def exp_pv(kcp, stw):
                    ex = ep.tile([128, 1024], bf16, tag="ex",
                                 name=f"exw{g}_{h}_{qh}_{kcp}")
                    nc.scalar.activation(ex, stw, Exp, scale=SCALE)
                    for j in (0, 1):
                        kc = kcp * 2 + j
                        nc.tensor.matmul(
                            pv, vt[(g, kc)][:, h * 97:(h + 1) * 97],
                            ex[:, j * 512:(j + 1) * 512],
                            start=(kc == 0), stop=(kc == 7))

                stws = []
                for kcp in range(4):
                    stw = ppq.tile([128, 1024], f32, tag="ps",
                                   name=f"stw{g}_{h}_{qh}_{kcp}")
                    for j in (0, 1):
                        kc = kcp * 2 + j
                        nc.tensor.matmul(
                            stw[:, j * 512:(j + 1) * 512],
                            kT[g][:, h * G + kc * 128:h * G + (kc + 1) * 128],
                            qs, start=True, stop=True)
                    stws.append(stw)
                    if kcp >= 1:
                        exp_pv(kcp - 1, stws[kcp - 1])
                exp_pv(3, stws[3])
                emit_ot(g, h, qh, pv)

            for blk in range(NB):
                g = blk // 4
                if blk % 4 == 0:
                    qT[g] = qtp.tile([HD, HPC * G], bf16, tag="qT",
                                     name=f"qT{g}")
                    kT[g] = ktp.tile([HD, HPC * G], bf16, tag="kT",
                                     name=f"kT{g}")
                ht, a_t, b_t = _pref.pop(blk)
                if blk + 1 < NB:
                    _pref[blk + 1] = fetch_blk(blk + 1)
                htv = ht.rearrange("p (k t) -> p k t", k=KC)

                for sub in range(2):
                    tb = blk * 2 + sub
                    col = (tb % 8) * 128
                    a_s = a_t[:, sub * CW:(sub + 1) * CW]
                    b_s = b_t[:, sub * CW:(sub + 1) * CW]
                    # one 2-bank psum: q = [0:288], v = [288:432]+[512:656],
                    # k = [656:944] (packed weight layout [wq | wv | wk])
                    ps = ppq.tile([128, 1024], f32, tag="ps",
                                  name=f"ps{tb}")
                    for kc in range(KC):
                        lhs = htv[:, kc, sub * 128:(sub + 1) * 128]
                        nc.tensor.matmul(ps[:, 0:432], lhs, w_kc(kc)[:, 0:432],
                                         start=(kc == 0), stop=(kc == KC - 1))
                        nc.tensor.matmul(ps[:, 512:944], lhs,
                                         w_kc(kc)[:, 432:864],
                                         start=(kc == 0), stop=(kc == KC - 1))
                    # V: bias add + interleaved ones column
                    v_t = vp.tile([128, HPC * (HD + 1)], bf16, tag="v",
                                  name=f"v{tb}")
                    vv = v_t.rearrange("p (h c) -> p h c", h=HPC)
                    bvv = bvi_sb.rearrange("p (h c) -> p h c", h=HPC)
                    nc.vector.tensor_tensor(vv[:, 0, 0:96], ps[:, 288:384],
                                            bvv[:, 0, 0:96], ADD)
                    nc.vector.tensor_tensor(vv[:, 1, 0:48], ps[:, 384:432],
                                            bvv[:, 1, 0:48], ADD)
                    nc.vector.tensor_tensor(vv[:, 1, 48:96], ps[:, 512:560],
                                            bvv[:, 1, 48:96], ADD)
                    nc.vector.tensor_tensor(vv[:, 2, 0:96], ps[:, 560:656],
                                            bvv[:, 2, 0:96], ADD)
                    nc.vector.tensor_copy(
                        vv[:, :, 96:97],
                        ones3.rearrange("p (h c) -> p h c", h=HPC))
                    vt[(g, tb % 8)] = v_t
                    # Q, K: bias, rope, transpose per head
                    for d, src0, bias in (("q", 0, bq_sb), ("k", 656, bk_sb)):
                        q_sb = qkp.tile([128, CW], bf16, tag=f"{d}sb",
                                        name=f"{d}sb{tb}")
                        nc.vector.tensor_tensor(q_sb, ps[:, src0:src0 + CW],
                                                bias, ADD)
                        shf = qkp.tile([128, CW], bf16, tag="shf",
                                       name=f"shf_{d}{tb}")
                        qv = q_sb.rearrange("p (c u f) -> p c u f", c=9, u=2)
                        sv = shf.rearrange("p (c u f) -> p c u f", c=9, u=2)
                        nc.vector.tensor_copy(sv[:, :, 0:1, :],
                                              qv[:, :, 1:2, :])
                        nc.vector.tensor_copy(sv[:, :, 1:2, :],
                                              qv[:, :, 0:1, :])
                        qr = qrp.tile([128, CW], bf16, tag="qr",
                                      name=f"qr_{d}{tb}")
                        nc.vector.tensor_tensor(shf, shf, b_s, MUL)
                        nc.vector.tensor_tensor(q_sb, q_sb, a_s, MUL)
                        nc.vector.tensor_tensor(qr, q_sb, shf, ADD)
                        dst = qT if d == "q" else kT
                        pt3 = pps.tile([HD, HPC * 128], bf16, tag="stpt",
                                       name=f"pt_{d}{tb}")
                        for h in range(HPC):
                            nc.tensor.transpose(
                                pt3[:, h * 128:(h + 1) * 128],
                                qr[:, h * 96:(h + 1) * 96], ident)
                        nc.vector.tensor_copy(
                            dst[g].rearrange("d (h t) -> d h t", h=HPC)
                            [:, :, col:col + 128],
                            pt3.rearrange("d (h t) -> d h t", h=HPC))
                    # drain one pending attention instance per sub-tile
                    if pending:
                        attn_instance(*pending.pop(0))
                if blk % 4 == 3:
                    pending.extend((g, h, qh)
                                   for h in range(HPC) for qh in range(2))
            while pending:
                attn_instance_wide(*pending.pop(0))
    nc.compile()
    return nc


def _build_launch2():
    import concourse.mybir as mybir
    import concourse.tile as tile
    from concourse import bacc

    f32 = mybir.dt.float32
    bf16 = mybir.dt.bfloat16
    TOK = 1024           # tokens per core
    NW = 1152            # outdims per core
    MB = NW // 128       # 9 outdim blocks
    nc = bacc.Bacc("TRN2", target_bir_lowering=False, debug=False)

    at_d = nc.dram_tensor("attnT", [DIM, TOK], bf16, kind="ExternalInput").ap()
    wo_d = nc.dram_tensor("woj", [DIM, NW], bf16, kind="ExternalInput").ap()
    bo_d = nc.dram_tensor("boj", [1, NW], f32, kind="ExternalInput").ap()
    # transposed output [outdim, tok]; host transposes back
    out_d = nc.dram_tensor("out", [NW, TOK], bf16, kind="ExternalOutput").ap()

    with tile.TileContext(nc) as tc:
        ats, wos = [], []
        with (
            tc.tile_pool(name="singles2", bufs=1) as singles,
            tc.tile_pool(name="atp", bufs=KC) as atp,
            tc.tile_pool(name="wop", bufs=KC) as wop,
            tc.tile_pool(name="outp", bufs=4) as outp,
            tc.tile_pool(name="psp", bufs=8, space="PSUM") as psp,
        ):
            bo_sb = singles.tile([128, MB], f32, tag="bo_sb", name="bo_sb")
            nc.gpsimd.dma_start(bo_sb,
                                bo_d.rearrange("a (m p) -> p (a m)", p=128))
            # input chunks interleaved across two queues in kc order; chunk 0
            # split in halves so the first matmul group starts sooner
            for kc in range(KC):
                a = atp.tile([128, TOK], bf16, tag="at", name=f"at{kc}")
                if kc == 0:
                    nc.sync.dma_start(a[:, 0:512], at_d[0:128, 0:512])
                    nc.sync.dma_start(a[:, 512:TOK], at_d[0:128, 512:TOK])
                else:
                    nc.sync.dma_start(a, at_d[kc * 128:(kc + 1) * 128, :])
                ats.append(a)
                w = wop.tile([128, NW], bf16, tag="wo", name=f"wo{kc}")
                if kc == 0:
                    nc.scalar.dma_start(w[:, 0:576], wo_d[0:128, 0:576])
                    nc.scalar.dma_start(w[:, 576:NW], wo_d[0:128, 576:NW])
                else:
                    nc.scalar.dma_start(w, wo_d[kc * 128:(kc + 1) * 128, :])
                wos.append(w)
            # chunk-outer accumulation over groups of 4 outdim blocks
            # (8 psum banks per group) so the PE tracks the DMA feed instead
            # of serializing behind it.
            units = [(mb, th) for mb in range(MB) for th in range(2)]
            ots = {}
            for base in range(0, len(units), 8):
                grp = units[base:base + 8]
                pss = {}
                for mb, th in grp:
                    pss[(mb, th)] = psp.tile([128, 512], f32, tag="ps",
                                             name=f"ps{mb}_{th}")
                for kc in range(KC):
                    for mb, th in grp:
                        nc.tensor.matmul(
                            pss[(mb, th)], wos[kc][:, mb * 128:(mb + 1) * 128],
                            ats[kc][:, th * 512:(th + 1) * 512],
                            start=(kc == 0), stop=(kc == KC - 1))
                for mb, th in grp:
                    if mb not in ots:
                        ots[mb] = outp.tile([128, TOK], bf16, tag="ot",
                                            name=f"ot{mb}")
                    nc.vector.tensor_scalar_add(
                        ots[mb][:, th * 512:(th + 1) * 512], pss[(mb, th)],
                        bo_sb[:, mb:mb + 1])
                    if th == 1:
                        nc.gpsimd.dma_start(out_d[mb * 128:(mb + 1) * 128, :],
                                            ots[mb])
    nc.compile()
    return nc


def _get(name, builder):
    if name not in _CACHE:
        _CACHE[name] = builder()
    return _CACHE[name]


def _rope_tables(frame, height, width):
    t = np.repeat(np.arange(frame), height * width)
    y = np.tile(np.repeat(np.arange(height), width), frame)
    x = np.tile(np.arange(width), frame * height)
    D = HD // 3
    A = np.empty((S, HD), np.float32)
    B = np.empty((S, HD), np.float32)
    for i, pos in enumerate((t, y, x)):
        inv = 1.0 / (10000.0 ** (np.arange(0, D, 2, dtype=np.float32) / D))
        f = pos[:, None].astype(np.float32) * inv[None, :]
        A[:, i * D:i * D + 16] = np.cos(f)
        A[:, i * D + 16:(i + 1) * D] = np.cos(f)
        B[:, i * D:i * D + 16] = -np.sin(f)
        B[:, i * D + 16:(i + 1) * D] = np.sin(f)
    return A, B


def _tile_hT(hT):
    # [2304, 4096] -> [NB, 128, KC*TB]: blk-major, partition-major, then
    # (chunk, token) contiguous per partition
    return np.ascontiguousarray(
        hT.reshape(KC, 128, NB, TB).transpose(2, 1, 0, 3).reshape(
            NB, 128, KC * TB))


def _tile_w(w):
    # [2304, PW] -> [128, KC*PW]
    return np.ascontiguousarray(
        w.reshape(KC, 128, PW).transpose(1, 0, 2).reshape(128, KC * PW))


def _tile_rope(a):
    # [4096, 288] (pre-tripled) -> [NB, 128, 2*288]
    return np.ascontiguousarray(
        a.reshape(NB, 2, 128, CW).transpose(0, 2, 1, 3).reshape(
            NB, 128, 2 * CW))


def kernel(hidden_states, wq, bq, wk, bk, wv, bv, wo, bo, frame, height, width):
    import ml_dtypes
    from concourse import bass_utils

    bf = ml_dtypes.bfloat16
    f, hh, ww = int(frame), int(height), int(width)
    hs = np.asarray(hidden_states, dtype=np.float32)
    assert hs.shape == (1, S, DIM) and f * hh * ww == S
    wq, wk, wv, wo = (np.asarray(a, np.float32) for a in (wq, wk, wv, wo))
    bq, bk, bv, bo = (np.asarray(a, np.float32) for a in (bq, bk, bv, bo))

    perm = np.concatenate([np.arange(k, S, SPN) for k in range(SPN)])
    A, B = _rope_tables(f, hh, ww)
    A = _tile_rope(np.tile(A[perm], (1, HPC))).astype(bf)
    B = _tile_rope(np.tile(B[perm], (1, HPC))).astype(bf)
    hT = _tile_hT(hs[0].T[:, perm]).astype(bf)

    nc1 = _get("l1", _build_launch1)
    in1 = []
    for c in range(8):
        sl = slice(c * CW, (c + 1) * CW)
        wqvk = np.concatenate([wq[:, sl], wv[:, sl], wk[:, sl]], axis=1)
        in1.append({
            "hT": hT,
            "w": _tile_w(wqvk).astype(bf),
            "bq": np.ascontiguousarray(bq[sl]).reshape(1, CW),
            "bk": np.ascontiguousarray(bk[sl]).reshape(1, CW),
            "bvi": np.concatenate(
                [np.concatenate([bv[sl][h * HD:(h + 1) * HD], [0.0]])
                 for h in range(HPC)]).astype(np.float32).reshape(1, -1),
            "A": A, "B": B,
        })
    td = os.environ.get("KERNEL_TRACE_DIR")
    if td:
        os.makedirs(td + "/l1", exist_ok=True)
        for fn in os.listdir(td + "/l1"):
            os.unlink(td + "/l1/" + fn)
    res1 = bass_utils.run_bass_kernel_spmd(
        nc1, in1, core_ids=list(range(8)),
        tmpdir=(td + "/l1") if td else None)
    LAST_RESULTS.append(res1)

    # outN [24, 97, 512] per core; slot = g*6 + h*2 + qh
    attnT_g = np.empty((DIM, S), np.float32)
    for c in range(8):
        o = np.asarray(res1.results[c]["outN"], dtype=np.float32)
        o = o.reshape(SPN, HPC, 2, HD + 1, 512)
        num = o[:, :, :, :HD, :]                  # [g, h, qh, 96, 512]
        den = o[:, :, :, HD:HD + 1, :]
        a = num / den                             # [g, h, qh, 96, 512]
        # -> rows (3c+h)*96 + d, cols g*1024 + qh*512 + t
        a = a.transpose(1, 3, 0, 2, 4).reshape(HPC * HD, S)
        attnT_g[c * CW:(c + 1) * CW, :] = a
    attnT = np.empty_like(attnT_g)
    attnT[:, perm] = attnT_g

    nc2 = _get("l2", _build_launch2)
    in2 = []
    attnT_bf = attnT.astype(bf)
    for c in range(8):
        i, j = divmod(c, 2)
        in2.append({
            "attnT": np.ascontiguousarray(attnT_bf[:, i * 1024:(i + 1) * 1024]),
            "woj": np.ascontiguousarray(
                wo[:, j * 1152:(j + 1) * 1152].astype(bf)),
            "boj": np.ascontiguousarray(
                bo[j * 1152:(j + 1) * 1152]).reshape(1, 1152),
        })
    if td:
        os.makedirs(td + "/l2", exist_ok=True)
        for fn in os.listdir(td + "/l2"):
            os.unlink(td + "/l2/" + fn)
    res2 = bass_utils.run_bass_kernel_spmd(
        nc2, in2, core_ids=list(range(8)),
        tmpdir=(td + "/l2") if td else None)
    LAST_RESULTS.append(res2)

    out = np.empty((S, DIM), np.float32)
    for c in range(8):
        i, j = divmod(c, 2)
        out[i * 1024:(i + 1) * 1024, j * 1152:(j + 1) * 1152] = \
            np.asarray(res2.results[c]["out"], dtype=np.float32).T
    return out[None]
